# revision 41
# baseline (speedup 1.0000x reference)
import sys

sys.path.insert(0, "/opt/trn_rl_repo")

import math

import numpy as np
import ml_dtypes

import concourse.bass as bass
import concourse.mybir as mybir
import concourse.tile as tile
from concourse import bacc
from concourse.bass_utils import run_bass_kernel_spmd
from concourse.masks import make_identity

F32 = mybir.dt.float32
F32R = mybir.dt.float32r
BF16 = mybir.dt.bfloat16
IDENT = mybir.ActivationFunctionType.Identity
EXPF = mybir.ActivationFunctionType.Exp

B, S, D = 8, 1024, 1024
N_H = 16
REL_K = 16
d_k = D // N_H  # 64
N_CORES = 8
MASKVAL = -1e30

# Score segments per key-block jt: absolute query ranges, each one matmul
# (<=512 cols; f32r needs >=256 cols for full rate).  First segment of each
# block carries the band add; jt7 is padded to 256 cols (768..896 computed
# but never exp'd).
SEGS = {
    0: [(0, 512), (512, 1024)],
    1: [(128, 512), (512, 1024)],
    2: [(256, 512), (512, 1024)],
    3: [(384, 768), (768, 1024)],
    4: [(512, 1024)],
    5: [(640, 1024)],
    6: [(768, 1024)],
    7: [(896, 1024)],
}

ZDP_SZ = 17 * 1040
EWD_BLK = 128 * 144
EWD_SZ = 8 * EWD_BLK
ZB2_BLK = 128 * 161
ZB2_SZ = 8 * ZB2_BLK
ZB1_BLK = 16 * 145
ZB1_SZ = 8 * ZB1_BLK

_CACHE = {}
TRACE = False


def _ap(t, offset, dims):
    return bass.AP(tensor=t, offset=offset, ap=[list(d) for d in dims])


def build_module():
    nc = bacc.Bacc("TRN2", detect_race_conditions=False, num_swdge_queues=4)

    xT = nc.dram_tensor("xT", [D, S], BF16, kind="ExternalInput")
    Wqk = nc.dram_tensor("Wqk", [D, 2 * D], BF16, kind="ExternalInput")
    Wv = nc.dram_tensor("Wv", [D, D], BF16, kind="ExternalInput")
    Wp = nc.dram_tensor("Wp", [D, D], BF16, kind="ExternalInput")
    bqk = nc.dram_tensor("bqk", [128, 16], F32, kind="ExternalInput")
    bvp = nc.dram_tensor("bvp", [1, D], BF16, kind="ExternalInput")
    bp = nc.dram_tensor("bp", [1, D], BF16, kind="ExternalInput")
    dlut = nc.dram_tensor("dlut", [d_k, 16], BF16, kind="ExternalInput")
    dlv = nc.dram_tensor("dlv", [16, d_k], BF16, kind="ExternalInput")
    selm2 = nc.dram_tensor("selm2", [2, 128], F32R, kind="ExternalInput")
    zb2c = [nc.dram_tensor(f"zb2c{i}", [ZB2_SZ], BF16, kind="ExternalInput")
            for i in range(2)]
    zb1c = [nc.dram_tensor(f"zb1c{i}", [ZB1_SZ], BF16, kind="ExternalInput")
            for i in range(2)]
    OUT = nc.dram_tensor("OUT", [S, D], F32, kind="ExternalOutput")

    zdp = [nc.dram_tensor(f"zdp{i}", [ZDP_SZ], BF16) for i in range(2)]
    ewd = [nc.dram_tensor(f"ewd{i}", [EWD_SZ], BF16) for i in range(2)]

    with tile.TileContext(nc) as tc, nc.allow_low_precision(
            reason="bf16 attention weights/corrections within tolerance"), \
            tc.tile_pool(name="persist", bufs=1) as pers:
        # ---- xT first: the first qkproj matmuls gate the whole pipeline ----
        xT_sb = []
        for d in range(8):
            t = pers.tile([128, S], BF16, name=f"xTt{d}", tag=f"xT{d}")
            nc.sync.dma_start(out=t[:], in_=xT[128 * d:128 * (d + 1), :])
            xT_sb.append(t)

        # ---- constants (scalar queue; keep SP free for xT) ----
        ident = pers.tile([128, 128], F32)
        make_identity(nc, ident[:])
        identB = pers.tile([128, 128], BF16)
        nc.vector.tensor_copy(identB[:], ident[:])
        dlut_sb = pers.tile([128, 16], BF16)
        nc.scalar.dma_start(out=dlut_sb[0:64, :], in_=dlut[:])
        nc.scalar.dma_start(out=dlut_sb[64:128, :], in_=dlut[:])
        dlv_sb = pers.tile([16, d_k], BF16)
        nc.scalar.dma_start(out=dlv_sb[:], in_=dlv[:])
        selm2_sb = pers.tile([2, 128], F32R)
        nc.scalar.dma_start(out=selm2_sb[:], in_=selm2[:])
        bqk_sb = pers.tile([128, 16], F32)
        nc.scalar.dma_start(out=bqk_sb[:], in_=bqk[:])
        bvp_sb = pers.tile([1, D], BF16)
        nc.scalar.dma_start(out=bvp_sb[:], in_=bvp[:])
        bp_sb = pers.tile([1, D], BF16)
        nc.scalar.dma_start(out=bp_sb[:], in_=bp[:])
        ones1 = pers.tile([1, 128], BF16)
        nc.vector.memset(ones1[:], 1.0)
        onescol = pers.tile([128, 16], BF16)
        nc.vector.memset(onescol[:], 1.0)

        dpT_t = [pers.tile([16, 1040], BF16, name=f"dpT{i}", tag=f"dpT{i}")
                 for i in range(2)]
        for i in range(2):
            nc.vector.memset(dpT_t[i][:, 1024:1040], 0.0)
        dpSh_t = [pers.tile([16, 1040], BF16, name=f"dpSh{i}", tag=f"dpSh{i}")
                  for i in range(2)]
        dpS_t = [pers.tile([128, 128], BF16, name=f"dpS{i}", tag=f"dpS{i}")
                 for i in range(2)]
        esT_t = [pers.tile([16, 1024], BF16, name=f"esT{i}", tag=f"esT{i}")
                 for i in range(2)]

        vhat_sb = [pers.tile([128, 16 * 65], BF16, name=f"vh{jt}",
                             tag=f"vh{jt}") for jt in range(8)]
        pair_sb = [pers.tile([128, S], BF16, name=f"pair{hp}",
                             tag=f"pair{hp}") for hp in range(8)]

        with (
            tc.tile_pool(name="wqkp", bufs=3) as wqkp,
            tc.tile_pool(name="qkp", bufs=3) as qkp,
            tc.tile_pool(name="bandp", bufs=4) as bandp,
            tc.tile_pool(name="ps_s", bufs=4, space="PSUM") as ps_s,
            tc.tile_pool(name="ps_tr", bufs=2, space="PSUM") as ps_tr,
        ):
            wq_tiles = {}
            qk_tiles = {}
            band_tiles = {}
            dt_tiles = {}
            den_tiles = {}

            def emit_wload(hp):
                ws = []
                for sec, ft in ((0, hp), (1, 8 + hp)):
                    w = wqkp.tile([128, 1024], BF16, name=f"wqk{hp}_{sec}",
                                  tag=f"wqk{sec}")
                    src = _ap(Wqk[:].tensor, 128 * ft,
                              [[2 * D, 128], [128 * 2 * D, 8], [1, 128]])
                    dst = _ap(w[:].tensor, 0,
                              [[1024, 128], [128, 8], [1, 128]])
                    nc.gpsimd.dma_start(out=dst, in_=src)
                    ws.append(w)
                wq_tiles[hp] = ws

            def qkproj_fillers(hp):
                ws = wq_tiles.pop(hp)
                pair = [qkp.tile([128, S], BF16, name=f"qk{hp}_0", tag="qk0"),
                        qkp.tile([128, S], BF16, name=f"qk{hp}_1", tag="qk1")]
                qk_tiles[hp] = pair

                def chunk(sec, ft, tch):
                    def emit():
                        ps = ps_s.tile([128, 512], F32, name="psqk", tag="s")
                        for d in range(8):
                            nc.tensor.matmul(
                                ps[:],
                                ws[sec][:, 128 * d:128 * (d + 1)],
                                xT_sb[d][:, 512 * tch:512 * (tch + 1)],
                                start=(d == 0), stop=(d == 7),
                            )
                        nc.scalar.activation(
                            pair[sec][:, 512 * tch:512 * (tch + 1)], ps[:],
                            IDENT, bias=bqk_sb[:, ft:ft + 1], scale=1.0)
                    return emit
                return [chunk(sec, ft, tch) for sec, ft in ((0, hp), (1, 8 + hp))
                        for tch in range(2)]

            def emit_qkproj(hp):
                for f in qkproj_fillers(hp):
                    f()

            def emit_band_stage1(h):
                hp, hh = h // 2, h % 2
                po = 64 * hh
                par = h % 2
                qT = qk_tiles[hp][0]
                for c in range(2):
                    psdp = ps_s.tile([128, 512], F32, name="psdp", tag="s")
                    nc.tensor.matmul(
                        psdp[0:16, :], dlut_sb[po:po + 64, :],
                        qT[po:po + 64, 512 * c:512 * (c + 1)],
                        start=True, stop=True)
                    nc.vector.tensor_copy(
                        dpT_t[par][:, 512 * c:512 * (c + 1)], psdp[0:16, :])
                nc.gpsimd.dma_start(
                    out=_ap(zdp[par][:].tensor, 0, [[1040, 16], [1, 1040]]),
                    in_=dpT_t[par][:])
                nc.sync.dma_start(
                    out=dpSh_t[par][:],
                    in_=_ap(zdp[par][:].tensor, 0, [[1041, 16], [1, 1040]]))

            def emit_band_stage2(h):
                par = h % 2
                psG = ps_tr.tile([128, 128], BF16, name="psG", tag="tr")
                for jt in range(8):
                    j0 = 128 * jt
                    nc.tensor.transpose(
                        psG[:, 16 * jt:16 * (jt + 1)],
                        dpSh_t[par][:, j0:j0 + 128],
                        identB[0:16, 0:16])
                nc.vector.tensor_copy(dpS_t[par][:], psG[:])
                nc.gpsimd.dma_start(
                    out=_ap(zb2c[par][:].tensor, 0,
                            [[162, 128], [ZB2_BLK, 8], [1, 16]]),
                    in_=_ap(dpS_t[par][:].tensor, 0,
                            [[128, 128], [16, 8], [1, 16]]))
                band = bandp.tile([128, 8 * 144], BF16, name=f"band{h}",
                                  tag="band")
                nc.sync.dma_start(
                    out=_ap(band[:].tensor, 0,
                            [[8 * 144, 128], [144, 8], [1, 144]]),
                    in_=_ap(zb2c[par][:].tensor, 0,
                            [[161, 128], [ZB2_BLK, 8], [1, 144]]))
                band_tiles[h] = band

            # ---- bootstrap: qkproj(0) + band stage1 before vproj ----
            emit_wload(0)
            emit_wload(1)
            emit_qkproj(0)
            emit_band_stage1(0)
            emit_band_stage1(1)

            # ---- v projection (PE-dense; hides band bootstrap latency) ----
            with tc.tile_pool(name="wv", bufs=1) as wvp:
                Wv_sb = []
                for d in range(8):
                    t = wvp.tile([128, D], BF16, name=f"wvt{d}", tag=f"wv{d}")
                    nc.gpsimd.dma_start(out=t[:],
                                        in_=Wv[128 * d:128 * (d + 1), :])
                    Wv_sb.append(t)
                for tt in range(8):
                    if tt == 4:
                        emit_band_stage2(0)
                        emit_band_stage2(1)
                    vt = vhat_sb[tt]
                    ones_ap = _ap(vt[:].tensor, 64, [[16 * 65, 128], [65, 16]])
                    nc.vector.tensor_copy(ones_ap, onescol[:])
                    for fc in range(2):
                        ps = ps_s.tile([128, 512], F32, name="psv", tag="s")
                        for d in range(8):
                            nc.tensor.matmul(
                                ps[:],
                                xT_sb[d][:, 128 * tt:128 * (tt + 1)],
                                Wv_sb[d][:, 512 * fc:512 * (fc + 1)],
                                start=(d == 0), stop=False,
                            )
                        nc.tensor.matmul(
                            ps[:], ones1[:], bvp_sb[:, 512 * fc:512 * (fc + 1)],
                            start=False, stop=True,
                        )
                        src = _ap(ps[:].tensor, ps[:].offset,
                                  [[512, 128], [64, 8], [1, 64]])
                        dst = _ap(vt[:].tensor, 65 * 8 * fc,
                                  [[16 * 65, 128], [65, 8], [1, 64]])
                        nc.scalar.copy(dst, src)

            # ---- attention-only pools + head loop ----
            with (
                tc.tile_pool(name="expp", bufs=3) as expp,
                tc.tile_pool(name="dtp", bufs=3) as dtp,
                tc.tile_pool(name="sm", bufs=3) as sm,
                tc.tile_pool(name="outtp", bufs=3) as outtp,
                tc.tile_pool(name="denp", bufs=2) as denp,
                tc.tile_pool(name="ps_o", bufs=1, space="PSUM") as ps_o,
            ):

                def emit_scores(h, fillers=()):
                    fillers = list(fillers)
                    hp, hh = h // 2, h % 2
                    po = 64 * hh
                    par = h % 2
                    qT = qk_tiles[hp][0]
                    kT = qk_tiles[hp][1]
                    band = band_tiles.pop(h)
                    eA = expp.tile([128, 4096], BF16, name=f"eA{h}", tag="eA")
                    eB = expp.tile([128, 4096], BF16, name=f"eB{h}", tag="eB")
                    pso = ps_o.tile([65, 1024], F32, name="pso", tag="pso")

                    def slot(jt):
                        return (eA, 1024 * jt) if jt < 4 else \
                            (eB, 1024 * (jt - 4))

                    pso_pending = []

                    def flush_pso(upto, final=False):
                        keep = [p for p in pso_pending if p[0] > upto]
                        todo = [p for p in pso_pending if p[0] <= upto]
                        for i, (jt2, a, b) in enumerate(todo):
                            j02 = 128 * jt2
                            t2, o2 = slot(jt2)
                            nc.tensor.matmul(
                                pso[:, a:b],
                                vhat_sb[jt2][:, 65 * h:65 * h + 65],
                                t2[:, o2 + a - j02:o2 + b - j02],
                                start=(jt2 == 0),
                                stop=(final and i == len(todo) - 1),
                                skip_group_check=True)
                        pso_pending.clear()
                        pso_pending.extend(keep)

                    for jt in range(8):
                        j0 = 128 * jt
                        t, o = slot(jt)
                        win = min(144, S - j0)
                        for si, (a, b) in enumerate(SEGS[jt]):
                            psS = ps_s.tile([128, 512], F32, name="psS",
                                            tag="s")
                            first = (si == 0)
                            nc.tensor.matmul(
                                psS[:, 0:b - a],
                                kT[po:po + 64, j0:j0 + 128],
                                qT[po:po + 64, a:b],
                                start=True, stop=not first,
                                skip_group_check=True)
                            if first:
                                c0 = 0
                                bw = b - a
                                nc.tensor.matmul(
                                    psS[:, c0:c0 + min(144, bw)], identB[:],
                                    band[:, 144 * jt:144 * jt + min(144, bw)],
                                    start=False, stop=True,
                                    skip_group_check=True)
                                nc.scalar.activation(
                                    t[:, o:o + bw], psS[:, c0:c0 + bw], EXPF)
                                if jt == 7:
                                    # pad cols 128..144 of the jt7 slot: the
                                    # ewd write reads 144 cols per slot
                                    nc.vector.memset(t[:, o + 128:o + 144],
                                                     0.0)
                            else:
                                nc.scalar.activation(
                                    t[:, o + a - j0:o + b - j0],
                                    psS[:, 0:b - a], EXPF)
                        lo = j0
                        if lo < 512:
                            pso_pending.append((jt, lo, 512))
                            pso_pending.append((jt, 512, 1024))
                        else:
                            pso_pending.append((jt, lo, 1024))
                        flush_pso(jt - 2)
                        npop = -(-len(fillers) // (8 - jt)) if jt < 7 else 0
                        for _ in range(min(npop, 2)):
                            fillers.pop(0)()
                        if jt == 3:
                            nc.gpsimd.dma_start(
                                out=_ap(ewd[par][:].tensor, 0,
                                        [[144, 128], [EWD_BLK, 4], [1, 144]]),
                                in_=_ap(eA[:].tensor, 0,
                                        [[4096, 128], [1024, 4], [1, 144]]))
                    flush_pso(7, final=True)
                    for f in fillers:
                        f()
                    nc.gpsimd.dma_start(
                        out=_ap(ewd[par][:].tensor, 4 * EWD_BLK,
                                [[144, 128], [EWD_BLK, 4], [1, 144]]),
                        in_=_ap(eB[:].tensor, 0,
                                [[4096, 128], [1024, 4], [1, 144]]))
                    return pso

                def emit_esk_read(h):
                    par = h % 2
                    esk = sm.tile([128, 128], BF16, name=f"esk{h}", tag="esk")
                    nc.sync.dma_start(
                        out=_ap(esk[:].tensor, 0,
                                [[128, 128], [16, 8], [1, 16]]),
                        in_=_ap(ewd[par][:].tensor, 0,
                                [[145, 128], [EWD_BLK, 8], [1, 16]]))
                    return esk

                def dt_back_half(h, esk, half):
                    par = h % 2
                    psE = ps_tr.tile([16, 512], BF16, name="psE", tag="tr")
                    for q in range(4):
                        jt = 4 * half + q
                        nc.tensor.transpose(
                            psE[:, 128 * q:128 * (q + 1)],
                            esk[:, 16 * jt:16 * (jt + 1)],
                            identB[:])
                    nc.vector.tensor_copy(
                        esT_t[par][:, 512 * half:512 * (half + 1)], psE[:])
                    if half == 1:
                        nc.gpsimd.dma_start(
                            out=_ap(zb1c[par][:].tensor, 0,
                                    [[146, 16], [ZB1_BLK, 8], [1, 128]]),
                            in_=_ap(esT_t[par][:].tensor, 0,
                                    [[1024, 16], [128, 8], [1, 128]]))
                        dt_all = dtp.tile([16, 8 * 144], BF16, name=f"dt{h}",
                                          tag="dt")
                        nc.sync.dma_start(
                            out=_ap(dt_all[:].tensor, 0,
                                    [[8 * 144, 16], [144, 8], [1, 144]]),
                            in_=_ap(zb1c[par][:].tensor, 0,
                                    [[145, 16], [ZB1_BLK, 8], [1, 144]]))
                        dt_tiles[h] = dt_all

                def emit_dt_back(h, esk):
                    dt_back_half(h, esk, 0)
                    dt_back_half(h, esk, 1)

                def dt_mm_quad(h, blo):
                    # key-block DT matmuls into a small psum tile + DVE adds
                    # straight into pair_sb rows (overlaps become plain adds)
                    hp, hh = h // 2, h % 2
                    ph = 64 * hh
                    dt_all = dt_tiles[h]
                    for jt in range(blo, blo + 4):
                        j0 = 128 * jt
                        win = min(144, S - j0)
                        psdt = ps_tr.tile([64, 144], F32, name="psdt",
                                          tag="tr")
                        nc.tensor.matmul(
                            psdt[:, 0:win], dlv_sb[:],
                            dt_all[:, 144 * jt:144 * jt + win],
                            start=True, stop=True)
                        nc.vector.tensor_add(
                            pair_sb[hp][ph:ph + 64, j0:j0 + win],
                            pair_sb[hp][ph:ph + 64, j0:j0 + win],
                            psdt[:, 0:win])
                    if blo == 4:
                        dt_tiles.pop(h)

                def normalize_pair(hp):
                    den2 = den_tiles.pop(hp)
                    recip2 = sm.tile([2, 1024], F32R, name=f"rc{hp}",
                                     tag="rc")
                    nc.vector.reciprocal(recip2[:], den2[:])
                    for c in range(2):
                        psb = ps_s.tile([128, 512], F32, name="psb", tag="s")
                        nc.tensor.matmul(
                            psb[:], selm2_sb[:],
                            recip2[:, 512 * c:512 * (c + 1)],
                            start=True, stop=True)
                        nc.vector.tensor_mul(
                            pair_sb[hp][:, 512 * c:512 * (c + 1)],
                            pair_sb[hp][:, 512 * c:512 * (c + 1)],
                            psb[:])

                def emit_dt_mms(h):
                    dt_mm_quad(h, 0)
                    dt_mm_quad(h, 4)
                    if h % 2 == 1:
                        normalize_pair(h // 2)

                def emit_evict(h, pso):
                    hp, hh = h // 2, h % 2
                    po = 64 * hh
                    outT = outtp.tile([65, 1024], BF16, name=f"outT{h}",
                                      tag="outT")
                    nc.vector.tensor_copy(outT[:], pso[:])
                    nc.sync.dma_start(out=pair_sb[hp][po:po + 64, :],
                                      in_=outT[0:64, :])
                    if hh == 0:
                        den_tiles[hp] = denp.tile([2, 1024], BF16,
                                                  name=f"den{hp}", tag="den")
                    nc.sync.dma_start(out=den_tiles[hp][hh:hh + 1, :],
                                      in_=outT[64:65, :])

                for h in range(16):
                    hp, hh = h // 2, h % 2
                    if hh == 0:
                        if hp + 2 < 8:
                            emit_wload(hp + 2)
                        if hp + 1 < 8:
                            emit_qkproj(hp + 1)
                            emit_band_stage1(2 * hp + 2)
                            emit_band_stage1(2 * hp + 3)
                    esk_prev = emit_esk_read(h - 1) if h >= 1 else None
                    pso = emit_scores(h)
                    if h + 2 < 16:
                        emit_band_stage2(h + 2)
                    if h >= 1:
                        emit_dt_back(h - 1, esk_prev)
                    if h >= 2:
                        emit_dt_mms(h - 2)
                    emit_evict(h, pso)
                    if hh == 1:
                        qk_tiles.pop(hp, None)

                esk15 = emit_esk_read(15)
                emit_dt_back(15, esk15)
                emit_dt_mms(14)
                emit_dt_mms(15)

            # ---- final projection ----
            with (
                tc.tile_pool(name="wp", bufs=1) as wpp,
                tc.tile_pool(name="outp", bufs=2) as outp,
            ):
                Wp_sb = []
                for d in range(8):
                    t = wpp.tile([128, D], BF16, name=f"wpt{d}", tag=f"wp{d}")
                    nc.gpsimd.dma_start(out=t[:],
                                        in_=Wp[128 * d:128 * (d + 1), :])
                    Wp_sb.append(t)
                for tt in range(8):
                    ot = outp.tile([128, 1024], F32, name="ot", tag="ot")
                    for fc in range(2):
                        ps = ps_s.tile([128, 512], F32, name="psp", tag="s")
                        for d in range(8):
                            nc.tensor.matmul(
                                ps[:],
                                pair_sb[d][:, 128 * tt:128 * (tt + 1)],
                                Wp_sb[d][:, 512 * fc:512 * (fc + 1)],
                                start=(d == 0), stop=False,
                            )
                        nc.tensor.matmul(
                            ps[:], ones1[:], bp_sb[:, 512 * fc:512 * (fc + 1)],
                            start=False, stop=True,
                        )
                        nc.vector.tensor_copy(
                            ot[:, 512 * fc:512 * (fc + 1)], ps[:])
                    nc.sync.dma_start(out=OUT[128 * tt:128 * (tt + 1), :],
                                      in_=ot[:])

    nc.compile()
    return nc


def _host_prep(W_attn, b_attn, W_proj, b_proj, lut_k, lut_v):
    scale = 1.0 / math.sqrt(d_k)
    Wqk = np.concatenate([W_attn[:, :D], W_attn[:, D:2 * D] * scale], axis=1)
    bq = b_attn[:D]
    bk = b_attn[D:2 * D] * scale
    bqk_h = np.stack([np.concatenate([bq, bk])[128 * ft:128 * (ft + 1)]
                      for ft in range(16)], axis=1).astype(np.float32)
    bvp_h = (b_attn[2 * D:3 * D] + np.tile(lut_v[0], N_H)).reshape(1, D)
    dlut_h = np.stack([(lut_k[16 - u] - lut_k[0]) * scale for u in range(16)],
                      axis=1).astype(np.float32)
    dlv_h = np.stack([lut_v[16 - u] - lut_v[0] for u in range(16)],
                     axis=0).astype(ml_dtypes.bfloat16)
    selm2_h = np.zeros((2, 128), np.float32)
    for p in range(128):
        selm2_h[p // 64, p] = 1.0
    blk = np.zeros((128, 161), np.float32)
    cols = np.arange(161)[None, :]
    rows = np.arange(128)[:, None]
    blk[cols < rows] = MASKVAL
    zb2c_h = np.tile(blk.reshape(-1), 8).astype(ml_dtypes.bfloat16)
    zb1c_h = np.zeros(ZB1_SZ, ml_dtypes.bfloat16)
    return {
        "Wqk": np.ascontiguousarray(Wqk).astype(ml_dtypes.bfloat16),
        "Wv": np.ascontiguousarray(W_attn[:, 2 * D:3 * D]).astype(ml_dtypes.bfloat16),
        "Wp": np.ascontiguousarray(W_proj).astype(ml_dtypes.bfloat16),
        "bqk": bqk_h,
        "bvp": np.ascontiguousarray(bvp_h).astype(ml_dtypes.bfloat16),
        "bp": np.ascontiguousarray(
            np.asarray(b_proj).reshape(1, D)).astype(ml_dtypes.bfloat16),
        "dlut": dlut_h.astype(ml_dtypes.bfloat16),
        "dlv": dlv_h,
        "selm2": selm2_h,
        "zb2c0": zb2c_h,
        "zb2c1": zb2c_h.copy(),
        "zb1c0": zb1c_h,
        "zb1c1": zb1c_h.copy(),
    }


def kernel(x, W_attn, b_attn, W_proj, b_proj, lut_k, lut_v):
    x = np.asarray(x, np.float32)
    shared = _host_prep(np.asarray(W_attn, np.float32),
                        np.asarray(b_attn, np.float32),
                        np.asarray(W_proj, np.float32),
                        np.asarray(b_proj, np.float32),
                        np.asarray(lut_k, np.float32),
                        np.asarray(lut_v, np.float32))
    if "nc" not in _CACHE:
        _CACHE["nc"] = build_module()
    nc = _CACHE["nc"]
    in_maps = []
    for b in range(N_CORES):
        m = dict(shared)
        m["xT"] = np.ascontiguousarray(x[b].T).astype(ml_dtypes.bfloat16)
        in_maps.append(m)
    res = run_bass_kernel_spmd(nc, in_maps, list(range(N_CORES)), trace=TRACE)
    _CACHE["last_result"] = res
    out = np.stack([res.results[b]["OUT"] for b in range(N_CORES)], axis=0)
    return out.astype(np.float32)
